# revision 24
# baseline (speedup 1.0000x reference)
"""Trainium2 Bass kernel for nn_AMLNeuralNetwork3D — row-split L1 variant.

L1 is ROW-split (contraction over the core's own 1024 genes), so it needs
no collective before it: the PE starts real work ~35us in, while the
CC-channel init barrier (~65us) completes in the background.  L1 produces
full-height partials [8192, NB] which are ReduceScatter'd (bf16) to the
core's feature slice, bias+relu applied, then AllGather'd — after which
L2/L3 proceed column-split exactly as the baseline.

Tail: the last chunk of L3 runs in two feature-half passes so the final
PSUM drain overlaps the second pass.
"""

import sys

if "/opt/trn_rl_repo" not in sys.path:
    sys.path.insert(0, "/opt/trn_rl_repo")

import numpy as np
import ml_dtypes

N_CORES = 8
G = 8192
B = 1024
L = 4
GS = G // N_CORES
NB = 512
NCHUNK = B // NB
GT = GS // 128    # 8 gene tiles per core slice / k-tiles for row-split L1
KT = G // 128     # 64 out-feature tiles for row-split L1 / k-tiles for L2,L3

BF16 = ml_dtypes.bfloat16

_compiled = {}

# gathered-feature order of the half-split L1 transition:
# [core0 f0:512, core1 f1024:1536, ...] then the second halves
_PERM_HALVES = np.concatenate(
    [np.arange(r * GS + a * 512, r * GS + (a + 1) * 512)
     for a in range(2) for r in range(N_CORES)]
)

N_WARMUP = 110


def _build_graph():
    from concourse import bacc, tile
    from concourse.tile_rust import add_dep_helper
    import concourse.mybir as mybir

    fp32 = mybir.dt.float32
    bf16 = mybir.dt.bfloat16
    Relu = mybir.ActivationFunctionType.Relu
    Copy = mybir.ActivationFunctionType.Copy
    mult = mybir.AluOpType.mult
    add = mybir.AluOpType.add
    bypass = mybir.AluOpType.bypass

    nc = bacc.Bacc(None, target_bir_lowering=False, num_devices=N_CORES)

    x_p = nc.declare_dram_parameter("x", [L, GS, B], bf16, isOutput=False)
    # per-feature scalars: cols 0..3 = W_local, 4 = b_local, 5..7 = b1..b3
    scal_p = nc.declare_dram_parameter("scal", [GS, 8], fp32, isOutput=False)
    # w1t: row-split tiled [m, p, g*128+c] = W1[m*128+c, own_slice_g*128+p]
    w1_p = nc.declare_dram_parameter("w1t", [KT, 128, GS], bf16, isOutput=False)
    # full b1 (all 8192 features), tiled [p, g] = b1[g*128+p]: L2's input
    # relu+bias is applied per gathered h tile instead of before the gather
    b1f_p = nc.declare_dram_parameter("b1f", [128, KT], fp32, isOutput=False)
    w_p = {
        k: nc.declare_dram_parameter(f"w{k}t", [G, GS], bf16, isOutput=False)
        for k in (2, 3)
    }
    out_p = nc.declare_dram_parameter("out", [GS, B], fp32, isOutput=True)

    rg = [list(range(N_CORES))]

    with tile.TileContext(nc) as tc:
        with (
            tc.tile_pool(name="dram", bufs=1, space="DRAM") as dram,
            tc.tile_pool(name="scal", bufs=GT) as spool,
            tc.tile_pool(name="xin", bufs=12) as xpool,
            tc.tile_pool(name="loc", bufs=6) as lpool,
            tc.tile_pool(name="h0p", bufs=2 * GT) as h0pool,
            tc.tile_pool(name="hin", bufs=32) as hpool,
            tc.tile_pool(name="wblk", bufs=24) as wpool,
            tc.tile_pool(name="hout", bufs=10) as opool,
            tc.tile_pool(name="pcp", bufs=24) as pcpool,
            tc.tile_pool(name="hact", bufs=16) as hapool,
            tc.tile_pool(name="psum", bufs=8, space="PSUM") as ppool,
        ):
            _gath_space = "Shared"
            # L1 partial buffers, feature-half split so the ReduceScatter of
            # half A starts at 50% of the chunk: part_X[j] row c*512 + q*128
            # holds features c*1024 + (X==B)*512 + q*128 (X in {A,B})
            part = [
                [
                    dram.tile([G // 2, NB], bf16, name=f"part{j}_{a}",
                              tag=f"part{j}_{a}")
                    for a in range(2)
                ]
                for j in range(NCHUNK)
            ]
            rsout = [
                [
                    dram.tile([GS // 2, NB], bf16, name=f"rsout{j}_{a}",
                              tag=f"rsout{j}_{a}")
                    for a in range(2)
                ]
                for j in range(NCHUNK)
            ]
            # gathered (pre-activation) L1 output, half-major feature order
            gath1 = [
                [
                    dram.tile(
                        [G // 2, NB], bf16, name=f"gath1_{j}_{a}",
                        tag=f"gath1_{j}_{a}", addr_space=_gath_space,
                    )
                    for a in range(2)
                ]
                for j in range(NCHUNK)
            ]
            # transitions 1,2 (after L1/L2): one AG per batch chunk
            slc = [
                [
                    dram.tile([GS, NB], bf16, name=f"slc_{t}_{j}", tag=f"slc_{t}_{j}")
                    for j in range(NCHUNK)
                ]
                for t in range(2)
            ]
            gath = [
                [
                    dram.tile(
                        [G, NB], bf16, name=f"gath_{t}_{j}", tag=f"gath_{t}_{j}",
                        addr_space=_gath_space,
                    )
                    for j in range(NCHUNK)
                ]
                for t in range(2)
            ]

            # --- PE warmup bridges until the local layer's h0 is ready
            wu_w = spool.tile([128, 128], bf16, name="wu_w", tag="wu_w")
            nc.sync.dma_start(wu_w[:], w_p[2][0:128, 0:128])
            wu_h = spool.tile([128, NB], bf16, name="wu_h", tag="wu_h")
            nc.sync.dma_start(wu_h[:], w_p[2][0:128, 0:NB])
            wu_ps = ppool.tile([128, NB], fp32, name="wu_ps", tag="ps")
            for i in range(N_WARMUP):
                nc.tensor.matmul(
                    wu_ps[:], wu_w[:], wu_h[:],
                    start=(i == 0), stop=(i == N_WARMUP - 1),
                )
            wu_out = spool.tile([128, NB], bf16, name="wu_out", tag="wu_out")
            nc.scalar.activation(wu_out[:], wu_ps[:], Copy)
            wu_dram = dram.tile([128, NB], bf16, name="wu_dram", tag="wu_dram")
            nc.scalar.dma_start(wu_dram[:], wu_out[:])

            sc = []
            for gt in range(GT):
                s = spool.tile([128, 8], fp32, name=f"sc{gt}", tag="sc")
                nc.sync.dma_start(s[:], scal_p[gt * 128 : (gt + 1) * 128, :])
                sc.append(s)
            b1f = spool.tile([128, KT], fp32, name="b1f", tag="b1f")
            nc.sync.dma_start(b1f[:], b1f_p[:, :])

            # ---- local layer on per-chunk [128, NB] tiles, chunk 0 first so
            # L1 reaches full rate as early as possible ----
            acts = []
            h0 = [[None] * GT for _ in range(NCHUNK)]
            for j in range(NCHUNK):
                for gt in range(GT):
                    xt = []
                    for l in range(L):
                        t = xpool.tile([128, NB], bf16, name=f"x{j}_{gt}_{l}",
                                       tag="x")
                        nc.sync.dma_start(
                            t[:],
                            x_p[l, gt * 128 : (gt + 1) * 128,
                                j * NB : (j + 1) * NB],
                        )
                        xt.append(t)
                    acc = lpool.tile([128, NB], bf16, name=f"a{j}_{gt}_0",
                                     tag="acc")
                    nc.vector.tensor_scalar(
                        acc[:], xt[0][:], sc[gt][:, 0:1], None, mult
                    )
                    for l in range(1, L):
                        acc2 = lpool.tile([128, NB], bf16,
                                          name=f"a{j}_{gt}_{l}", tag="acc")
                        nc.vector.scalar_tensor_tensor(
                            acc2[:], xt[l][:], sc[gt][:, l : l + 1], acc[:],
                            mult, add
                        )
                        acc = acc2
                    h = h0pool.tile([128, NB], bf16, name=f"h0_{j}_{gt}",
                                    tag="h0")
                    ai = nc.scalar.activation(h[:], acc[:], Relu,
                                              bias=sc[gt][:, 4:5])
                    if j == 0:
                        acts.append(ai)
                    h0[j][gt] = h

            # m-tile order: all feature-half-A tiles (m%8 < 4) first, so the
            # half-A ReduceScatter+AllGather runs while half B computes
            M_ORDER = [m for m in range(KT) if m % 8 < 4] + [
                m for m in range(KT) if m % 8 >= 4
            ]

            def l1_rowsplit(j):
                # partial[m*128+c, b] = sum_g w1t[m][:, g] . h0[g][:, chunk j]
                wdmas = {}
                NPREF = 8
                for i in range(NPREF):
                    m = M_ORDER[i]
                    wb = wpool.tile([128, GS], bf16, name=f"w1_{j}_{m}", tag="wblk")
                    wdma = nc.sync.dma_start(wb[:], w1_p[m, :, :])
                    if j == 0:
                        add_dep_helper(
                            getattr(wdma, "ins", wdma),
                            getattr(acts[1], "ins", acts[1]),
                            reason="x tiles first on HBM",
                        )
                    wdmas[m] = wb
                for i in range(KT):
                    m = M_ORDER[i]
                    if i + NPREF < KT:
                        mq = M_ORDER[i + NPREF]
                        wb = wpool.tile(
                            [128, GS], bf16, name=f"w1_{j}_{mq}", tag="wblk"
                        )
                        nc.sync.dma_start(wb[:], w1_p[mq, :, :])
                        wdmas[mq] = wb
                    wb = wdmas.pop(m)
                    pst = ppool.tile([128, NB], fp32, name=f"ps1_{j}_{m}", tag="ps")
                    for g in range(GT):
                        nc.tensor.matmul(
                            pst[:],
                            wb[:, g * 128 : (g + 1) * 128],
                            h0[j][g][:],
                            start=(g == 0),
                            stop=(g == GT - 1),
                        )
                    # PSUM-freeing copy on the otherwise-idle Vector engine;
                    # the part write goes via Scalar, where a backlog (when a
                    # concurrent ReduceScatter hogs the DMA engines) blocks
                    # only further part writes — pcpool depth is the elasticity
                    pc = pcpool.tile([128, NB], bf16, name=f"pc{j}_{m}", tag="pcp")
                    nc.vector.tensor_scalar(pc[:], pst[:], 1.0, None, mult)
                    a = 0 if m % 8 < 4 else 1
                    row = (m // 8) * 512 + (m % 4) * 128
                    nc.scalar.dma_start(part[j][a][row : row + 128, :], pc[:])
                    if i == KT // 2 - 1:
                        rs_ag(j, 0)
                rs_ag(j, 1)

            cc_chain = []

            def chain(cc):
                # force GpSimd trigger order = emission order so the serial
                # CC stream can't reorder (a late RS ahead of a ready AG)
                if cc_chain:
                    add_dep_helper(
                        getattr(cc, "ins", cc),
                        getattr(cc_chain[-1], "ins", cc_chain[-1]),
                        reason="cc stream order",
                    )
                cc_chain.append(cc)

            def rs_ag(j, a):
                # ReduceScatter the raw partials of feature-half a, then
                # AllGather the raw reduced slice immediately; bias+relu is
                # applied on the gathered tiles as L2 loads them
                cc = nc.gpsimd.collective_compute(
                    "ReduceScatter", add, replica_groups=rg,
                    ins=[part[j][a][:].opt()], outs=[rsout[j][a][:].opt()],
                )
                chain(cc)
                cc = nc.gpsimd.collective_compute(
                    "AllGather", bypass, replica_groups=rg,
                    ins=[rsout[j][a][:].opt()], outs=[gath1[j][a][:].opt()],
                )
                chain(cc)

            def h_dma(k, j, g, ht):
                if k == 2:
                    # gathered L1 output is half-major (w2t rows permuted)
                    src = gath1[j][g // (KT // 2)]
                    row = (g % (KT // 2)) * 128
                    return nc.sync.dma_start(ht[:], src[row : row + 128, :])
                src = gath[k - 2][j]
                return nc.sync.dma_start(ht[:], src[g * 128 : (g + 1) * 128, :])

            def dense_layer(k, j):
                # k in {2,3}; input from gath[k-2][j]
                wt = w_p[k]
                ps = [
                    ppool.tile([128, NB], fp32, name=f"ps{k}_{j}_{o}", tag="ps")
                    for o in range(GT)
                ]
                wdmas = {}
                NPREF = 8
                for g in range(NPREF):
                    wb = wpool.tile([128, GS], bf16, name=f"w{k}_{j}_{g}", tag="wblk")
                    nc.sync.dma_start(wb[:], wt[g * 128 : (g + 1) * 128, :])
                    wdmas[g] = wb
                for g in range(KT):
                    ht = hpool.tile([128, NB], bf16, name=f"h{k}_{j}_{g}", tag="hin")
                    h_dma(k, j, g, ht)
                    if g + NPREF < KT:
                        gq = g + NPREF
                        wb = wpool.tile(
                            [128, GS], bf16, name=f"w{k}_{j}_{gq}", tag="wblk"
                        )
                        nc.sync.dma_start(wb[:], wt[gq * 128 : (gq + 1) * 128, :])
                        wdmas[gq] = wb
                    if k == 2:
                        # h is the raw (pre-activation) gathered L1 output
                        ha = hapool.tile(
                            [128, NB], bf16, name=f"ha{j}_{g}", tag="hact"
                        )
                        nc.scalar.activation(
                            ha[:], ht[:], Relu, bias=b1f[:, g : g + 1]
                        )
                        ht = ha
                    wb = wdmas.pop(g)
                    for o in range(GT):
                        nc.tensor.matmul(
                            ps[o][:],
                            wb[:, o * 128 : (o + 1) * 128],
                            ht[:],
                            start=(g == 0),
                            stop=(g == KT - 1),
                        )
                for o in range(GT):
                    if k < 3:
                        ot = opool.tile(
                            [128, NB], bf16, name=f"o{k}_{j}_{o}", tag="hout"
                        )
                        nc.scalar.activation(
                            ot[:], ps[o][:], Relu, bias=sc[o][:, 4 + k : 5 + k]
                        )
                        nc.sync.dma_start(
                            slc[k - 1][j][o * 128 : (o + 1) * 128, :], ot[:]
                        )
                    else:
                        ot = opool.tile(
                            [128, NB], fp32, name=f"o{k}_{j}_{o}", tag="outp"
                        )
                        nc.scalar.activation(
                            ot[:], ps[o][:], Relu, bias=sc[o][:, 7:8]
                        )
                        nc.sync.dma_start(
                            out_p[o * 128 : (o + 1) * 128, j * NB : (j + 1) * NB],
                            ot[:],
                        )

            def dense_layer_last(k, j):
                wt = w_p[k]
                for half in range(2):
                    ps = [
                        ppool.tile([128, NB], fp32, name=f"ps{k}_{j}_{half}_{o}",
                                   tag="ps")
                        for o in range(GT // 2)
                    ]
                    col = half * (GS // 2)
                    for g in range(KT):
                        ht = hpool.tile(
                            [128, NB], bf16, name=f"h{k}_{j}_{half}_{g}", tag="hin"
                        )
                        h_dma(k, j, g, ht)
                        wb = wpool.tile(
                            [128, GS // 2], bf16, name=f"w{k}_{j}_{half}_{g}",
                            tag="wblk",
                        )
                        nc.sync.dma_start(
                            wb[:], wt[g * 128 : (g + 1) * 128, col : col + GS // 2]
                        )
                        for o in range(GT // 2):
                            nc.tensor.matmul(
                                ps[o][:],
                                wb[:, o * 128 : (o + 1) * 128],
                                ht[:],
                                start=(g == 0),
                                stop=(g == KT - 1),
                            )
                    for o in range(GT // 2):
                        oo = half * (GT // 2) + o
                        ot = opool.tile(
                            [128, NB], fp32, name=f"o{k}_{j}_{half}_{o}", tag="outp"
                        )
                        nc.scalar.activation(
                            ot[:], ps[o][:], Relu, bias=sc[oo][:, 7:8]
                        )
                        nc.sync.dma_start(
                            out_p[oo * 128 : (oo + 1) * 128,
                                  j * NB : (j + 1) * NB],
                            ot[:],
                        )

            def allgather(t, j):
                cc = nc.gpsimd.collective_compute(
                    "AllGather", bypass, replica_groups=rg,
                    ins=[slc[t - 1][j][:].opt()], outs=[gath[t - 1][j][:].opt()],
                )
                chain(cc)

            # emission order = desired overlap order
            l1_rowsplit(0)
            l1_rowsplit(1)
            for j in range(NCHUNK):
                dense_layer(2, j)
                allgather(2, j)
            dense_layer(3, 0)
            dense_layer_last(3, 1)

    nc.compile()
    return nc


def _get_nc():
    if "nc" not in _compiled:
        _compiled["nc"] = _build_graph()
    return _compiled["nc"]


def kernel(x, W_local, b_local, W1, b1, W2, b2, W3, b3):
    from concourse.bass_utils import run_bass_kernel_spmd

    nc = _get_nc()

    x = np.asarray(x)
    W1 = np.asarray(W1)
    in_maps = []
    for r in range(N_CORES):
        sl = slice(r * GS, (r + 1) * GS)
        x_r = x[:, :, sl].transpose(0, 2, 1).astype(BF16)
        scal_r = np.concatenate(
            [
                np.asarray(W_local)[sl, :],
                np.asarray(b_local)[sl, None],
                np.asarray(b1)[sl, None],
                np.asarray(b2)[sl, None],
                np.asarray(b3)[sl, None],
            ],
            axis=1,
        ).astype(np.float32)
        # row-split L1 weight tiles: [m, p, g*128+c] = W1[m*128+c, own g*128+p]
        w1s = W1[:, sl].reshape(KT, 128, GT, 128).transpose(0, 3, 2, 1)
        in_maps.append(
            {
                "x": x_r,
                "scal": np.ascontiguousarray(scal_r),
                "w1t": np.ascontiguousarray(w1s.reshape(KT, 128, GS)).astype(BF16),
                "b1f": np.ascontiguousarray(
                    np.asarray(b1)[_PERM_HALVES].reshape(KT, 128).T
                ).astype(np.float32),
                "w2t": np.asarray(W2)[sl, :].T.astype(BF16)[_PERM_HALVES, :],
                "w3t": np.asarray(W3)[sl, :].T.astype(BF16),
            }
        )

    res = run_bass_kernel_spmd(nc, in_maps, core_ids=list(range(N_CORES)))

    out = np.empty((B, G), np.float32)
    for r in range(N_CORES):
        out[:, r * GS : (r + 1) * GS] = res.results[r]["out"].T
    return out
